# revision 86
# baseline (speedup 1.0000x reference)
"""BigBird simulated attention on 8 Trainium2 NeuronCores.

Strategy
--------
B*H = 24 (batch, head) pairs are sharded 3-per-core across 8 cores (data/head
parallel). The BigBird mask is block-constant on 64x64 tiles, so the host
compresses it to a 64x64 block map and bakes a block-sparse schedule directly
into the instruction stream (the mask never goes to the device).

Per (head, q-block of 64 rows) scores are computed TRANSPOSED (S^T: k on
partitions, q on free) so the exp'd probabilities are directly the stationary
operand of the PV matmul -- no on-chip transposes:

  S^T[k, q] = sum_d K[k, d] Q[q, d]    (lhsT = K^T block cols, rhs = Q^T)
  P^T = exp(S^T / 8)                    (one ScalarE activation per wave)
  acc[q, :] = sum_k P^T[k, q]^T Vaug[k, :]    with Vaug = [V | 1]

The ones-column of Vaug makes acc[:, 64] the softmax denominator, so the
normalization is one reciprocal + per-partition-scalar multiply at the end.
Max-subtraction is skipped: scores are ~N(0,1) after scaling, exp cannot
overflow, and softmax is shift-invariant. All matmul operands are bf16
(tolerance is 2e-2; measured error ~6e-3); PSUM accumulation stays fp32.

Q-blocks are processed in PAIRS that share most of their attended k-blocks,
so one QK matmul streams both blocks' q columns (128 moving cols) and one
PV matmul fills both accumulators (128 out partitions: low half = first
block, high half = second block):

  pair 0       = (0, 63): the global rows, fully dense -- 32 shared aligned
                 k-pair chunks, V from the resident natural-chunk tensor v2.
  pair i >= 1  = (2i-1, 2i): shared window chunk (k blocks 2i-1, 2i;
                 V from the 64-row-shifted resident v2s), shared global
                 chunk (k blocks 0+63, strided lhsT; V from resident vg),
                 and per-block private chunks: the leftover window block +
                 3 random blocks, packed two-per-chunk with a strided
                 dual-block lhsT (V pairs host-gathered and streamed, vp).

Private chunks stream only the owner's 64 q columns and accumulate into the
owner's 64-partition half. Every LDWEIGHTS is ~53 ns (128-col bf16 weights
get the compiler's fast-weight-load); the schedule has 218 chunks per head,
each one QK + one PV molecule.

All engines' instructions are chained in emission order with no-sync edges:
the list scheduler otherwise interleaves independent instructions across
waves, which breaks the one-wait observer discipline below (engines execute
in order anyway, so the chain is free).

The many 1x1 "observer" matmuls / nops exist because every engine instruction
on this toolchain carries at most ONE hardware sync wait: each observer
brings one engine up to date with one foreign semaphore so no real
instruction ever needs two waits.
"""

import ml_dtypes
import numpy as np

import concourse.bass as bass
import concourse.tile as tile
from concourse import mybir
from concourse.bass_utils import run_bass_kernel_spmd
from concourse.tile_rust import add_dep_helper

B, H, S, D = 2, 12, 4096, 64
BLK = 64
NB = S // BLK            # 64 blocks per axis
DA = D + 1               # v plus ones column
NCORES = 8
HPC = B * H // NCORES    # heads per core
SCALE = 1.0 / 8.0        # 1/sqrt(64)
WAVE_UNITS = 23          # 23*64 score cols used of a 3-bank tile; tail corner
NPAIRS = NB // 2         # 32 q-block pairs
OPB = 7                  # pairs per psum output bank (7*65 = 455 <= 512)
NBANK = (NPAIRS + OPB - 1) // OPB
NCHUNK = S // 128        # natural 128-row chunks of V
NSTRAD = NPAIRS - 1      # straddle chunks (2i-1, 2i), i = 1..31
FIN_BUFS = 6             # om ring depth: > banks/head so no in-head recycle

F32 = mybir.dt.float32
BF16 = mybir.dt.bfloat16
NP_BF16 = ml_dtypes.bfloat16


# ----------------------------------------------------------------- schedule

def _block_mask(mask: np.ndarray) -> np.ndarray:
    m = np.asarray(mask).reshape(NB, BLK, NB, BLK)
    bm = m[:, 0, :, 0]
    assert bool(np.all(m == bm[:, None, :, None])), (
        "mask is not 64x64 block-constant; this kernel's schedule requires it"
    )
    return bm > 0


def _pairs():
    return [(0, NB - 1)] + [(2 * i - 1, 2 * i) for i in range(1, NPAIRS)]


def _build_schedule(bm: np.ndarray):
    """Flat chunk list in waves.

    Entry: (pi, kind, payload, units, fst, lst, sidx)
      kind "SA": payload t       -- aligned k-pair (2t, 2t+1), pair 0 only
      kind "SW": payload i       -- window straddle k blocks (2i-1, 2i)
      kind "SG": payload None    -- global k blocks (0, 63)
      kind "PD": payload (par, gA, gB)  -- private dual (gB None = filler)
    units: score columns / 64 (2 for shared, 1 for private).
    sidx: running index into the host-gathered V-pair tensor (or -1).
    """
    pairs = _pairs()
    flat = []
    ns = 0
    # Bank-grouped order: per bank of OPB pairs, all shared chunks first
    # (SW then SG, SGs of consecutive pairs adjacent so emit_qk can merge
    # them into one wide matmul), then all private chunks. Privates land
    # late, so the kp/vp loads have slack; the last pair of each bank
    # still finishes last (the finalize trigger relies on it).
    for b0 in range(0, NPAIRS, OPB):
        group = list(range(b0, min(b0 + OPB, NPAIRS)))
        ent = []
        for pi in group:
            a, bb = pairs[pi]
            if pi == 0:
                La = set(np.nonzero(bm[a])[0].tolist())
                Lb = set(np.nonzero(bm[bb])[0].tolist())
                assert La == set(range(NB)) and Lb == set(range(NB))
                for t in range(NB // 2):
                    ent.append([pi, "SA", t, 2, False, False, -1])
            else:
                ent.append([pi, "SW", pi, 2, False, False, -1])
        for pi in group:
            if pi != 0:
                ent.append([pi, "SG", None, 2, False, False, -1])
        for pi in group:
            a, bb = pairs[pi]
            if pi == 0:
                continue
            La = set(np.nonzero(bm[a])[0].tolist())
            Lb = set(np.nonzero(bm[bb])[0].tolist())
            assert {a, bb, 0, NB - 1} <= La and {a, bb, 0, NB - 1} <= Lb
            for par, blk in ((0, a), (1, bb)):
                L = sorted((La if par == 0 else Lb) - {a, bb, 0, NB - 1})
                for k in range(0, len(L), 2):
                    gA = L[k]
                    gB = L[k + 1] if k + 1 < len(L) else None
                    ent.append([pi, "PD", (par, gA, gB), 1, False, False, ns])
                    ns += 1
        first_seen, last_idx = set(), {}
        for idx, e in enumerate(ent):
            if e[0] not in first_seen:
                first_seen.add(e[0])
                e[4] = True
            last_idx[e[0]] = idx
        for pi2, idx in last_idx.items():
            ent[idx][5] = True
        flat.extend(tuple(e) for e in ent)
    waves = []
    cur, units = [], 0
    for e in flat:
        if units + e[3] > WAVE_UNITS:
            waves.append(cur)
            cur, units = [], 0
        cur.append(e)
        units += e[3]
    if cur:
        waves.append(cur)
    # 128-col chunks first in each wave (stable): they start 128-aligned,
    # so their PSUM output never straddles a 2KB bank boundary at a
    # non-128-aligned offset (observed to corrupt the post-boundary half).
    # PV order within a wave is free: all PVs are pure accumulations.
    for wv in waves:
        wv.sort(key=lambda e: 0 if e[3] == 2 else 1)
    return waves, ns


# ------------------------------------------------------------------ program

def _crumb_cell2(rec, crumb):
    c = rec["ncr"]
    rec["ncr"] += 1
    assert rec["ncr"] <= 512
    return crumb[c // 512: c // 512 + 1, c % 512: c % 512 + 1]


def _ord2(rec, key, instr):
    """Pin same-engine emission order with a no-sync edge.

    The scheduler may interleave independent same-engine instructions
    across waves, which breaks the one-wait observer discipline (an exp's
    PE deps must all be covered by its a1 observer; the tail nops must
    observe each engine's true last instruction). Engines execute their
    queues in order anyway, so pinning emission order is free.
    """
    if rec.get(key) is not None:
        add_dep_helper(instr.ins, rec[key].ins, sync=False,
                       reason=f"{key} order chain")
    rec[key] = instr
    return instr


def _prefetch_head(tc, pools, h, qT_d, kT_d, v2_d, v2s_d, vg_d, kg_d,
                   qg_d, rec):
    """Issue head h's input loads; all heads are prefetched at program
    start (pools hold every head) so transfers overlap earlier compute."""
    nc = tc.nc
    wq, wk, wv = pools[0], pools[1], pools[2]
    qT = wq.tile([64, S], BF16, tag="qT", name=f"qT{h}")
    kT = wk.tile([64, S], BF16, tag="kT", name=f"kT{h}")
    v2 = wv.tile([128, NCHUNK * DA], BF16, tag="v2", name=f"v2_{h}")
    v2s = wv.tile([128, NSTRAD * DA], BF16, tag="v2s", name=f"v2s_{h}")
    vg = wv.tile([128, DA], BF16, tag="vg", name=f"vg_{h}")
    kg = wv.tile([64, 128], BF16, tag="kg", name=f"kg_{h}")
    qg = wv.tile([64, 128], BF16, tag="qg", name=f"qg_{h}")
    # wave-0 inputs (kT, qg, v2) first so head 0 starts computing early
    ds = [
        _ord2(rec, "pool_last", nc.gpsimd.dma_start(out=kT, in_=kT_d[h])),
        _ord2(rec, "pool_last", nc.gpsimd.dma_start(out=qg, in_=qg_d[h])),
        _ord2(rec, "pool_last", nc.gpsimd.dma_start(out=v2, in_=v2_d[h])),
        _ord2(rec, "pool_last", nc.gpsimd.dma_start(out=qT, in_=qT_d[h])),
        _ord2(rec, "pool_last", nc.gpsimd.dma_start(out=v2s, in_=v2s_d[h])),
        _ord2(rec, "pool_last", nc.gpsimd.dma_start(out=vg, in_=vg_d[h])),
        _ord2(rec, "pool_last", nc.gpsimd.dma_start(out=kg, in_=kg_d[h])),
    ]
    rec["dmas"] += ds
    return (qT, kT, v2, v2s, vg, kg, qg, ds)


def _load_kp(tc, kpp, h, ns, kp_d, consts, rec):
    """Whole-head kp load, double-buffered, issued a full head ahead.
    On slot reuse (h >= 2) Pool first observes PE (past the old
    tenant's QK readers) and the old loads. 4 slices so the earliest
    private chunks unblock as soon as their slice lands."""
    nc = tc.nc
    cst, a1out, crumb = consts
    kp = kpp.tile([64, ns * 128], BF16, tag="kp", name=f"kp_{h}")
    pins = []
    if h >= 2:
        pabs = _ord2(rec, "pool_last", nc.gpsimd.tensor_copy(
            _crumb_cell2(rec, crumb), cst[0:1, 3:4]))
        add_dep_helper(pabs.ins, rec["pe"].ins, sync=True,
                       reason="Pool observes PE before kp reuse")
        pins.append(pabs)
        for old in rec["kpdmas"][h - 2]:
            p2 = _ord2(rec, "pool_last", nc.gpsimd.tensor_copy(
                _crumb_cell2(rec, crumb), cst[0:1, 4:5]))
            add_dep_helper(p2.ins, old.ins, sync=True,
                           reason="Pool observes old kp load")
            pins.append(p2)
    dmas = []
    nsl = 4
    bounds = [ns * i // nsl for i in range(nsl + 1)]
    for i in range(nsl):
        s0, s1 = bounds[i], bounds[i + 1]
        dmas.append(_ord2(rec, "pool_last", nc.gpsimd.dma_start(
            out=kp[:, s0 * 128: s1 * 128].rearrange(
                "p (s c) -> p s c", c=128),
            in_=kp_d[h, s0: s1].rearrange("s p c -> p s c"))))
    for p in pins:
        for dm in dmas:
            add_dep_helper(dm.ins, p.ins, sync=False,
                           reason="absorbers precede kp load")
    rec["dmas"] += dmas
    rec["kpbuf"][h] = kp
    rec["kpdmas"][h] = tuple(dmas)


def _load_vp(tc, vpp, h, ns, vp_d, rec):
    """Whole-head vp load; triple-buffered so every head's slot is
    fresh and all loads issue at program start (no absorbers)."""
    nc = tc.nc
    vp = vpp.tile([128, ns * DA], BF16, tag="vp", name=f"vp_{h}")
    nsl = 4
    bounds = [ns * i // nsl for i in range(nsl + 1)]
    for i in range(nsl):
        s0, s1 = bounds[i], bounds[i + 1]
        rec["dmas"].append(_ord2(rec, "pool_last", nc.gpsimd.dma_start(
            out=vp[:, s0 * DA: s1 * DA].rearrange(
                "p (s c) -> p s c", c=DA),
            in_=vp_d[h, s0: s1].rearrange("s p c -> p s c"))))
    rec["vpbuf"][h] = vp


def _emit_head(tc, pools, h, waves, head_tiles, vp_d, o_d, consts, rec):
    nc = tc.nc
    wq, wk, wv, vppool, ppool, stpool, obpool, fpool = pools
    cst, a1out, crumb = consts

    qT, kT, v2, v2s, vg, kg, qg, head_dmas = head_tiles
    kp, vp = rec["kpbuf"][h], rec["vpbuf"][h]

    started = set()
    ob_tiles = {}
    fin_due = []
    CB = WAVE_UNITS * BLK           # scratch corner base col in score tiles

    def _chain(mm):
        rec["pe"] = _ord2(rec, "pe_last", mm)
        return mm

    def _cv(instr):                       # DVE order chain
        return _ord2(rec, "dve_last", instr)

    def _cp(instr):                       # Pool/GpSimd order chain
        return _ord2(rec, "pool_last", instr)

    def _ca(instr):                       # ACT order chain
        return _ord2(rec, "act_last", instr)

    def _dummy_mm(dst, src_ap):
        return _chain(nc.tensor.matmul(
            dst, lhsT=src_ap, rhs=src_ap,
            start=True, stop=True, skip_group_check=True))

    def _crumb_cell():
        c = rec["ncr"]
        rec["ncr"] += 1
        assert rec["ncr"] <= 512
        return crumb[c // 512: c // 512 + 1, c % 512: c % 512 + 1]

    def _open_ob(bank, scorner):
        # d1 targets the NEW bank's corner: a start=True write into a
        # still-accumulating bank would clear its has_written bits for
        # the written partitions; here d2's full reset follows anyway.
        ob = obpool.tile([128, 512], F32, tag="ob", name=f"obh{h}_{bank}")
        d1 = None
        if rec["om"] is not None:
            d1 = _dummy_mm(ob[0:1, 459:460], rec["om"])  # PE observes DVE
        # Bank-open reset: start=True clears the bank's has_written bits
        # for the OUT PARTITIONS it writes, bank-wide. Writing all 128
        # partitions here means every later PV (all start=False) will
        # overwrite the reused slot's stale data on its first touch of
        # each element and accumulate afterwards. A start=True on a real
        # PV instead would wipe the accumulation state of every other
        # in-flight pair in the bank.
        d2 = _chain(nc.tensor.matmul(
            ob[0:128, 460:461], lhsT=qg[0:1, 0:128], rhs=qg[0:1, 0:1],
            start=True, stop=True, skip_group_check=True))
        if d1 is not None:
            add_dep_helper(d2.ins, d1.ins, sync=False,
                           reason="DVE observer before fresh-bank touch")
        ob_tiles[bank] = ob
        return ob

    def _finalize(bank):
        ob = ob_tiles.pop(bank)
        j0 = bank * OPB
        nq = min(OPB, NPAIRS - j0)
        gbank = len(rec["odma"])
        if gbank >= FIN_BUFS:
            # om slot recycles: bring DVE up to date with both readers of
            # the old tenant (out-DMA and the Pool crumb copy). FIN_BUFS
            # exceeds one head's bank count, so the old tenant belongs to
            # the previous head and these waits are long satisfied.
            dabs = _cv(nc.vector.tensor_copy(_crumb_cell(), cst[0:1, 2:3]))
            add_dep_helper(dabs.ins, rec["odma"][gbank - FIN_BUFS].ins,
                           sync=True, reason="DVE observes om slot release")
            dabs2 = _cv(nc.vector.tensor_copy(_crumb_cell(), cst[0:1, 2:3]))
            add_dep_helper(dabs2.ins, rec["pool_hist"][gbank - FIN_BUFS].ins,
                           sync=True, reason="DVE observes om Pool reader")
        om = fpool.tile([128, OPB * BLK], BF16, tag="om",
                        name=f"omh{h}_{bank}")
        rcp = fpool.tile([128, 1], F32, tag="rcp", name=f"rcph{h}_{bank}")
        for j in range(nq):
            _cv(nc.vector.reciprocal(
                rcp[0:128, :], ob[0:128, j * DA + D: j * DA + D + 1]))
            rec["dve"] = _cv(nc.vector.tensor_scalar_mul(
                om[0:128, j * BLK: (j + 1) * BLK],
                ob[0:128, j * DA: j * DA + D],
                rcp[0:128, :]))
        rec["pool"] = _cp(nc.gpsimd.tensor_copy(
            out=_crumb_cell(),
            in_=om[0:1, (nq - 1) * BLK: (nq - 1) * BLK + 1]))
        rec["pool_hist"].append(rec["pool"])
        dma = _cp(nc.gpsimd.dma_start(
            out=o_d[h][bank][:, :].rearrange("(j p) c -> p j c", p=128),
            in_=om[0:128, : nq * BLK].rearrange("p (j c) -> p j c", c=BLK)))
        rec["dmas"].append(dma)
        rec["odma"].append(dma)
        # last-written om region: a RAW dep on it covers every ob read above
        rec["om"] = om[0:1, (nq - 1) * BLK: (nq - 1) * BLK + 1]

    def emit_qk(wave, st):
        c0 = 0
        ci = 0
        while ci < len(wave):
            (pi, kind, payload, units, fst, lst, sidx) = wave[ci]
            if kind == "SA":
                t = payload
                _chain(nc.tensor.matmul(
                    st[:, c0: c0 + 128],
                    lhsT=kT[0:64, 2 * t * BLK: (2 * t + 2) * BLK],
                    rhs=qg[0:64, 0:128],
                    start=True, stop=True, skip_group_check=True))
            elif kind == "SW":
                i = payload
                _chain(nc.tensor.matmul(
                    st[:, c0: c0 + 128],
                    lhsT=kT[0:64, (2 * i - 1) * BLK: (2 * i + 1) * BLK],
                    rhs=qT[0:64, (2 * i - 1) * BLK: (2 * i + 1) * BLK],
                    start=True, stop=True, skip_group_check=True))
            elif kind == "SG":
                # merge the run of SG chunks for consecutive pairs into
                # one wide matmul (their q columns are contiguous)
                run = 1
                SGM_MAX = int(__import__("os").environ.get("BB_SGM", 4))
                while (run < SGM_MAX
                       and ci + run < len(wave)
                       and wave[ci + run][1] == "SG"
                       and wave[ci + run][0] == pi + run):
                    run += 1
                _chain(nc.tensor.matmul(
                    st[:, c0: c0 + run * 128],
                    lhsT=kg[0:64, 0:128],
                    rhs=qT[0:64, (2 * pi - 1) * BLK:
                           (2 * (pi + run - 1) + 1) * BLK],
                    start=True, stop=True, skip_group_check=True))
                c0 += run * 2 * BLK
                ci += run
                continue
            else:                       # PD
                par, gA, gB = payload
                own = 2 * pi - 1 + par
                _chain(nc.tensor.matmul(
                    st[:, c0: c0 + BLK],
                    lhsT=kp[0:64, sidx * 128: (sidx + 1) * 128],
                    rhs=qT[0:64, own * BLK: (own + 1) * BLK],
                    start=True, stop=True, skip_group_check=True))
            c0 += units * BLK
            ci += 1

    def emit_pv(wave, pT, w):
        cur = max(ob_tiles)
        # start=False: a start=True matmul clears the WHOLE bank's
        # has_written bits (hardware), which would wipe the in-flight
        # accumulation of a pair split across waves in this bank
        _chain(nc.tensor.matmul(
            ob_tiles[cur][0:1, 461:462], lhsT=pT[0:1, 0:1],
            rhs=pT[0:1, 0:1], start=False, stop=True, skip_group_check=True))
        c0 = 0
        for (pi, kind, payload, units, fst, lst, sidx) in wave:
            bank = pi // OPB
            if bank not in ob_tiles:
                prev_ob = ob_tiles[max(ob_tiles)]
                _open_ob(bank, prev_ob[0:1, 462:463])
            ob = ob_tiles[bank]
            slot = (pi % OPB) * DA
            if kind == "SA":
                rhs = v2[:, payload * DA: (payload + 1) * DA]
            elif kind == "SW":
                rhs = v2s[:, (payload - 1) * DA: payload * DA]
            elif kind == "SG":
                rhs = vg[:, 0: DA]
            else:
                rhs = vp[:, sidx * DA: (sidx + 1) * DA]
            if kind == "PD":
                par = payload[0]
                osl = ob[par * 64: par * 64 + 64, slot: slot + DA]
                lhsT = pT[:, c0: c0 + BLK]
            else:
                osl = ob[0:128, slot: slot + DA]
                lhsT = pT[:, c0: c0 + 128]
            # start=False everywhere: _open_ob's corner dummy ran
            # start=True at bank open, clearing the whole bank's
            # has_written bits; a start=False matmul then WRITES the
            # first touch of each element (bit clear) and accumulates
            # after (bit set). A start=True here would wipe the bits of
            # every other in-flight pair in this bank.
            _chain(nc.tensor.matmul(
                osl, lhsT=lhsT, rhs=rhs,
                start=False, stop=lst, skip_group_check=True))
            if lst and (pi == (bank + 1) * OPB - 1 or pi == NPAIRS - 1):
                fin_due.append(bank)
            c0 += units * BLK
        newest = max(ob_tiles)
        for bank in [b for b in fin_due if b != newest]:
            fin_due.remove(bank)
            _finalize(bank)

    # ---- head preamble: first score tile is the corner target for the
    # preamble observers (benign: these writes precede its exp)
    st0 = stpool.tile([128, WAVE_UNITS * BLK + BLK], F32, tag="st",
                      name=f"sth{h}_0")
    _dummy_mm(st0[0:1, CB: CB + 1], cst[0:1, 0:1])           # PE drain
    # only wave-0's inputs gate the head start; the rest are observed
    # after wave 0's QK (emitted below) so their DMAs overlap compute
    for di, src in enumerate((kT, qg, v2)):                  # DMA queues
        _dummy_mm(st0[0:1, CB + 1 + di: CB + 2 + di], src[0:1, 0:1])
    if rec["om"] is not None:                                # DVE (prev head)
        _dummy_mm(st0[0:1, CB + 9: CB + 10], rec["om"])
    _open_ob(0, st0[0:1, CB + 10: CB + 11])

    # ---- software-pipelined waves ----
    prev = None
    for w, wave in enumerate(waves):
        if w == 0:
            st = st0
        else:
            st = stpool.tile([128, WAVE_UNITS * BLK + BLK], F32, tag="st",
                             name=f"sth{h}_{w}")
            _dummy_mm(st[0:1, CB: CB + 1], cst[0:1, 0:1])
        emit_qk(wave, st)
        if w == 0:
            for di, src in enumerate((qT, v2s, vg, kg)):     # late DMA qs
                _dummy_mm(st[0:1, CB + 4 + di: CB + 5 + di], src[0:1, 0:1])
        a1 = _ca(nc.scalar.activation(
            out=a1out[0:1, 0:1], in_=st[0:1, 0:1],
            func=mybir.ActivationFunctionType.Copy))
        add_dep_helper(a1.ins, rec["pe"].ins, sync=True,
                       reason="ACT observes PE after wave QK")
        rec["act"] = a1
        pT = ppool.tile([128, WAVE_UNITS * BLK], BF16, tag="pT",
                        name=f"pTh{h}_{w}")
        ncols = sum(e[3] for e in wave) * BLK
        rec["act"] = _ca(nc.scalar.activation(
            out=pT[:, :ncols], in_=st[:, :ncols],
            func=mybir.ActivationFunctionType.Exp, scale=SCALE))
        if prev is not None:
            emit_pv(prev[1], prev[2], prev[0])
        prev = (w, wave, pT)
    emit_pv(prev[1], prev[2], prev[0])
    for bank in list(fin_due):
        fin_due.remove(bank)
        _finalize(bank)
    for bank in sorted(ob_tiles):
        _finalize(bank)


def _build_program(bm: np.ndarray):
    import os as _os
    hpc = int(_os.environ.get("BB_HPC", HPC))
    nwaves = int(_os.environ.get("BB_NWAVES", 0))
    waves, ns = _build_schedule(bm)
    if nwaves:
        waves = waves[:nwaves]
    nc = bass.Bass("TRN2", target_bir_lowering=False, debug=False,
                   enable_asserts=False)
    qT_d = nc.dram_tensor("qT", [HPC, 64, S], BF16, kind="ExternalInput")
    kT_d = nc.dram_tensor("kT", [HPC, 64, S], BF16, kind="ExternalInput")
    v2_d = nc.dram_tensor("v2", [HPC, 128, NCHUNK * DA], BF16,
                          kind="ExternalInput")
    v2s_d = nc.dram_tensor("v2s", [HPC, 128, NSTRAD * DA], BF16,
                           kind="ExternalInput")
    vg_d = nc.dram_tensor("vg", [HPC, 128, DA], BF16, kind="ExternalInput")
    kg_d = nc.dram_tensor("kg", [HPC, 64, 128], BF16, kind="ExternalInput")
    qg_d = nc.dram_tensor("qg", [HPC, 64, 128], BF16, kind="ExternalInput")
    vp_d = nc.dram_tensor("vp", [HPC, ns, 128, DA], BF16,
                          kind="ExternalInput")
    kp_d = nc.dram_tensor("kp", [HPC, ns, 64, 128], BF16,
                          kind="ExternalInput")
    o_d = [[nc.dram_tensor(f"o_{hh}_{bb}",
                           [min(OPB, NPAIRS - bb * OPB) * 128, D], BF16,
                           kind="ExternalOutput")
            for bb in range(NBANK)] for hh in range(HPC)]

    with tile.TileContext(nc) as tc:
        with (
            tc.tile_pool(name="wq", bufs=HPC) as wq,
            tc.tile_pool(name="wk", bufs=HPC) as wk,
            tc.tile_pool(name="wv", bufs=HPC) as wv,
            tc.tile_pool(name="vpp", bufs=HPC) as vppool,
            tc.tile_pool(name="kpp", bufs=2) as kppool,
            tc.tile_pool(name="pT", bufs=2) as ppool,
            tc.tile_pool(name="st", bufs=2, space="PSUM") as stpool,
            tc.tile_pool(name="ob", bufs=2, space="PSUM") as obpool,
            tc.tile_pool(name="fin", bufs=FIN_BUFS) as fpool,
            tc.tile_pool(name="cstp", bufs=1) as cpool,
        ):
            pools = (wq, wk, wv, vppool, ppool, stpool, obpool, fpool)
            # bf16: a fp32 matmul (even a 1x1 dummy) runs as LOW/HIGH
            # halves and the HIGH half disables fast-weight-load on the
            # next LDWEIGHTS (hardware workaround), so observers must not
            # be fp32
            cst = cpool.tile([128, 8], BF16, tag="cst", name="cst")
            a1out = cpool.tile([128, 4], F32, tag="a1out", name="a1out")
            crumb = cpool.tile([128, 512], F32, tag="crumb", name="crumb")
            rec = {"dmas": [], "odma": [], "om": None, "ncr": 0,
                   "pool_hist": [], "kpbuf": {}, "kpdmas": {}, "vpbuf": {}}
            consts = (cst, a1out, crumb)
            _ord2(rec, "dve_last", tc.nc.vector.memset(cst, 0.0))
            _ord2(rec, "pool_last", tc.nc.gpsimd.tensor_copy(
                _crumb_cell2(rec, crumb), cst[0:1, 6:7]))
            head_tiles = []
            for hh in range(hpc):
                head_tiles.append(_prefetch_head(
                    tc, pools, hh, qT_d, kT_d, v2_d, v2s_d, vg_d, kg_d,
                    qg_d, rec))
                _load_vp(tc, vppool, hh, ns, vp_d, rec)
                if hh < 2:
                    _load_kp(tc, kppool, hh, ns, kp_d, consts, rec)
            for hh in range(hpc):
                if 1 <= hh < hpc - 1:
                    _load_kp(tc, kppool, hh + 1, ns, kp_d, consts, rec)
                _emit_head(tc, pools, hh, waves, head_tiles[hh], None, o_d,
                           consts, rec)
            # SP runs nothing; feed it one-wait nops covering each proc so
            # the framework's tail drain has no unobserved semaphores left
            tail = [rec[k] for k in ("pe", "act", "dve", "pool") if k in rec]
            tail += rec["dmas"][-16:]
            for td in tail:
                nop = tc.nc.sync.nop(nofuse=True)
                add_dep_helper(nop.ins, td.ins, sync=True,
                               reason="SP observes proc before tail drain")
    return nc


_CACHE = {}


def _get_program(bm: np.ndarray):
    key = bm.tobytes()
    if key not in _CACHE:
        _CACHE[key] = _build_program(bm)
    return _CACHE[key]


# -------------------------------------------------------------------- entry

def _prep_inputs(q, k, v, waves, ns):
    q = np.asarray(q, dtype=np.float32)
    k = np.asarray(k, dtype=np.float32)
    v = np.asarray(v, dtype=np.float32)
    qT = np.ascontiguousarray(
        q.reshape(B * H, S, D).transpose(0, 2, 1)).astype(NP_BF16)
    kT = np.ascontiguousarray(
        k.reshape(B * H, S, D).transpose(0, 2, 1)).astype(NP_BF16)
    vA = np.concatenate(
        [v.reshape(B * H, S, D),
         np.ones((B * H, S, 1), dtype=np.float32)], axis=2).astype(NP_BF16)
    v2 = np.ascontiguousarray(
        vA.reshape(B * H, NCHUNK, 128, DA).transpose(0, 2, 1, 3)
        .reshape(B * H, 128, NCHUNK * DA))
    # straddle chunks: V rows (2i-1)*64 .. (2i+1)*64 for i = 1..31
    v2s = np.ascontiguousarray(
        vA[:, 64: 64 + NSTRAD * 128].reshape(B * H, NSTRAD, 128, DA)
        .transpose(0, 2, 1, 3).reshape(B * H, 128, NSTRAD * DA))
    # global pair: V blocks 0 and 63 stacked; K/Q blocks 0 and 63 packed
    vg = np.ascontiguousarray(np.concatenate(
        [vA[:, 0:BLK], vA[:, S - BLK: S]], axis=1))     # [BH, 128, DA]
    kg = np.ascontiguousarray(np.concatenate(
        [kT[:, :, 0:BLK], kT[:, :, S - BLK: S]], axis=2))  # [BH, 64, 128]
    qg = np.ascontiguousarray(np.concatenate(
        [qT[:, :, 0:BLK], qT[:, :, S - BLK: S]], axis=2))  # [BH, 64, 128]
    # gather V/K pairs for private chunks (absent halves stay zero)
    vp = np.zeros((B * H, ns, 128, DA), dtype=NP_BF16)
    kp = np.zeros((B * H, ns, 64, 128), dtype=NP_BF16)
    vblk = vA.reshape(B * H, NB, BLK, DA)
    for wave in waves:
        for (pi, kind, payload, units, fst, lst, sidx) in wave:
            if kind != "PD":
                continue
            par, gA, gB = payload
            vp[:, sidx, 0:64, :] = vblk[:, gA]
            kp[:, sidx, :, 0:64] = kT[:, :, gA * BLK: (gA + 1) * BLK]
            if gB is not None:
                vp[:, sidx, 64:128, :] = vblk[:, gB]
                kp[:, sidx, :, 64:128] = kT[:, :, gB * BLK: (gB + 1) * BLK]
    return qT, kT, v2, v2s, vg, kg, qg, vp, kp


def _run(inputs, trace=False):
    q, k, v, mask = inputs["q"], inputs["k"], inputs["v"], inputs["mask"]
    bm = _block_mask(mask)
    nc = _get_program(bm)
    waves, ns = _build_schedule(bm)
    qT, kT, v2, v2s, vg, kg, qg, vp, kp = _prep_inputs(q, k, v, waves, ns)
    in_maps = []
    for c in range(NCORES):
        sl = slice(c * HPC, (c + 1) * HPC)
        in_maps.append({
            "qT": np.ascontiguousarray(qT[sl]),
            "kT": np.ascontiguousarray(kT[sl]),
            "v2": np.ascontiguousarray(v2[sl]),
            "v2s": np.ascontiguousarray(v2s[sl]),
            "vg": np.ascontiguousarray(vg[sl]),
            "kg": np.ascontiguousarray(kg[sl]),
            "qg": np.ascontiguousarray(qg[sl]),
            "vp": np.ascontiguousarray(vp[sl]),
            "kp": np.ascontiguousarray(kp[sl]),
        })
    bkr = run_bass_kernel_spmd(nc, in_maps, list(range(NCORES)), trace=trace)
    pairs = _pairs()
    out = np.empty((B * H, S, D), dtype=np.float32)
    for c, r in enumerate(bkr.results):
        for hh in range(HPC):
            gh = c * HPC + hh
            for bb in range(NBANK):
                piece = np.asarray(r[f"o_{hh}_{bb}"]).astype(np.float32)
                nq = min(OPB, NPAIRS - bb * OPB)
                piece = piece.reshape(nq, 2, BLK, D)
                for s in range(nq):
                    a, b2 = pairs[bb * OPB + s]
                    out[gh, a * BLK: (a + 1) * BLK] = piece[s, 0]
                    out[gh, b2 * BLK: (b2 + 1) * BLK] = piece[s, 1]
    return out.reshape(B, H, S, D), bkr


def kernel(**inputs):
    out, _ = _run(inputs, trace=False)
    return out
